# revision 1
# baseline (speedup 1.0000x reference)
"""MultiHeadLatentAttention on 8 Trainium2 NeuronCores (Bass/Tile, SPMD).

Sharding (tensor parallel over heads, per the hint, plus two refinements):
  - 16 heads / 8 cores = 2 heads per core: q_proj + kv_b_proj output dims and
    o_proj input dim sharded by head.
  - kv_a_proj + rms-norm are token-sharded (512 tokens/core) with an
    AllGather of the normalized latent (small: 1 MB/core) instead of
    replicating the 9.7 GFLOP kv_a matmul on every core.
  - Instead of an AllReduce of full [B,S,H] partial o_proj outputs (33 MB,
    ~380 us), an AllToAll of the attention outputs (4 MB) token-shards the
    o_proj: each core computes the full o_proj for 512 tokens and outputs
    exactly its token slice. Host-side unshard is a pure concat/transpose.

All matmuls run as fp32r (TF32: 10-bit mantissa inputs, fp32 accumulate) at
1 cycle/row on the PE. Inputs are pre-rounded to the TF32 grid on the host;
intermediates are rounded by the producing engine writing float32r.

Layouts keep tokens on the moving/free axis everywhere:
  hsT [hid, tok], qT/kT/vT [d, tok] per (head, batch), scoresT [ktok, qtok]
  (softmax along partitions via ones-matmul), attention out [d, tok],
  o_proj out [out, tok] (host transposes at the end).
"""

import math
from contextlib import ExitStack

import numpy as np

B, S = 2, 2048
T = B * S                     # 4096 flattened tokens
HID = 2048
H, D = 16, 128
RANK, ROPE = 512, 64
MAX_POS, ORIG_POS = 131072, 8192
BASE = 500000.0
BETA_FAST, BETA_SLOW = 32.0, 1.0
EPS = 1e-6
NCORES = 8
HPC = H // NCORES             # 2 heads per core
TPC = T // NCORES             # 512 tokens per core (kv_a shard)
SPC = S // NCORES             # 256 tokens per (core, batch) after AllToAll

_CACHE: dict = {}


def tf32_round(x: np.ndarray) -> np.ndarray:
    u = np.ascontiguousarray(x, dtype=np.float32).view(np.uint32).copy()
    add = ((u >> 13) & 1) + 0xFFF
    u = (u + add) & np.uint32(0xFFFFE000)
    return u.view(np.float32)


def _yarn_cos_sin():
    """cos/sin tables matching reference.py's yarn_cos_sin (mscale folded)."""
    scaling = MAX_POS / ORIG_POS
    pos_freqs = BASE ** (np.arange(0, ROPE, 2, dtype=np.float64) / ROPE)
    extrap = 1.0 / pos_freqs
    interp = 1.0 / (scaling * pos_freqs)
    low = max(math.floor(ROPE * math.log(ORIG_POS / (BETA_FAST * 2 * math.pi))
                         / (2 * math.log(BASE))), 0)
    high = min(math.ceil(ROPE * math.log(ORIG_POS / (BETA_SLOW * 2 * math.pi))
                         / (2 * math.log(BASE))), ROPE - 1)
    i = np.arange(ROPE // 2, dtype=np.float64)
    smooth = np.clip((i - low) / max(high - low, 1), 0.0, 1.0)
    inv_freq = ((1.0 - smooth) * interp + smooth * extrap).astype(np.float32)
    pos = np.arange(S, dtype=np.float32)
    freqs = pos[:, None] * inv_freq[None, :]              # [S, 32]
    emb = np.concatenate([freqs, freqs], axis=-1)         # [S, 64]
    mscale = 0.1 * math.log(scaling) + 1.0
    cos = (np.cos(emb) * mscale).astype(np.float32)
    sin = (np.sin(emb) * mscale).astype(np.float32)
    return cos.T.copy(), sin.T.copy()                     # [64, S] each


def build_nc(passes=1, sim_mode=False):
    """Build + compile the (single, SPMD) Bass program for all 8 cores."""
    import concourse.tile as tile
    import concourse.mybir as mybir
    from concourse import bacc

    F32 = mybir.dt.float32
    F32R = mybir.dt.float32r
    AF = mybir.ActivationFunctionType
    RG = [list(range(NCORES))]

    nc = bacc.Bacc("TRN2", target_bir_lowering=False, debug=False,
                   num_devices=1 if sim_mode else NCORES)

    # ---- kernel I/O ----
    hsT_in = nc.dram_tensor("hsT", [HID, T], F32R, kind="ExternalInput").ap()
    hsmy_in = nc.dram_tensor("hsmy", [HID, TPC], F32R, kind="ExternalInput").ap()
    qwT_in = nc.dram_tensor("qwT", [HID, HPC * D], F32R, kind="ExternalInput").ap()
    kvaT_in = nc.dram_tensor("kvaT", [HID, RANK], F32R, kind="ExternalInput").ap()
    kvbT_in = nc.dram_tensor("kvbT", [RANK, HPC * 2 * D], F32R, kind="ExternalInput").ap()
    owt_in = nc.dram_tensor("owt", [16, 128, HID], F32R, kind="ExternalInput").ap()
    cos_in = nc.dram_tensor("cos", [ROPE, S], F32, kind="ExternalInput").ap()
    sinsh_in = nc.dram_tensor("sinsh", [ROPE, S], F32, kind="ExternalInput").ap()
    ident_in = nc.dram_tensor("ident", [128, 128], F32R, kind="ExternalInput").ap()
    ones_in = nc.dram_tensor("ones", [128, 128], F32R, kind="ExternalInput").ap()
    outTs = [nc.dram_tensor(f"outT{p}" if p else "outT", [HID, 2 * SPC], F32,
                            kind="ExternalOutput").ap() for p in range(passes)]

    NH = HID // 128   # 16 hid chunks
    NR = RANK // 128  # 4 rank chunks

    with tile.TileContext(nc) as tc, ExitStack() as ctx0:
        const = ctx0.enter_context(tc.tile_pool(name="const", bufs=1))
        dram = ctx0.enter_context(tc.tile_pool(name="dram", bufs=1, space="DRAM"))

        ident = const.tile([128, 128], F32R)
        ones = const.tile([128, 128], F32R)
        cosb = const.tile([ROPE, S], F32)
        sinsh = const.tile([ROPE, S], F32)
        eps_t = const.tile([1, 1], F32)
        nc.sync.dma_start(ident[:], ident_in[:])
        nc.sync.dma_start(ones[:], ones_in[:])
        nc.sync.dma_start(cosb[:], cos_in[:])
        nc.sync.dma_start(sinsh[:], sinsh_in[:])
        nc.vector.memset(eps_t[:], EPS)

        for p_ in range(passes):
            # collective bounce buffers
            ag_in = [dram.tile([RANK // 2, TPC], F32R, name=f"agin{p_}{h}")
                     for h in range(2)]
            ag_out = [dram.tile([NCORES, RANK // 2, TPC], F32R,
                                addr_space="Local" if sim_mode else "Shared",
                                name=f"agout{p_}{h}") for h in range(2)]
            a2a_in = [dram.tile([NCORES, HPC * D, SPC], F32R, name=f"a2ain{p_}{b}")
                      for b in range(B)]
            a2a_out = [dram.tile([NCORES, HPC * D, SPC], F32R, name=f"a2aout{p_}{b}")
                       for b in range(B)]

            ctx_pass = ExitStack()
            afp = ctx_pass.enter_context(tc.tile_pool(name=f"afp_{p_}", bufs=1))
            af = afp.tile([128, NH * 2 * SPC], F32R, name=f"af{p_}")
            with ExitStack() as ctx_big:
                big = ctx_big.enter_context(tc.tile_pool(name=f"big_{p_}", bufs=1))
                rope_pool = ctx_big.enter_context(
                    tc.tile_pool(name=f"rope_{p_}", bufs=1))

                def rope_block(X):
                    tmp = rope_pool.tile([ROPE, S], F32, tag="rtmp", bufs=1,
                                         name="rtmp")
                    m2 = rope_pool.tile([ROPE, S], F32, tag="rm2", bufs=1,
                                        name="rm2")
                    nc.vector.tensor_mul(tmp[:], X[0:ROPE], cosb[:])
                    nc.vector.tensor_mul(m2[0:32], X[32:64], sinsh[32:64])
                    nc.vector.tensor_mul(m2[32:64], X[0:32], sinsh[0:32])
                    nc.vector.tensor_add(X[0:ROPE], tmp[:], m2[:])

                # per (head j, batch b) tiles, [128, S] each
                qT = [[big.tile([128, S], F32R, name=f"qT{p_}{j}{b}") for b in range(B)]
                      for j in range(HPC)]
                kT = [[big.tile([128, S], F32R, name=f"kT{p_}{j}{b}") for b in range(B)]
                      for j in range(HPC)]
                vnat = [[big.tile([128, S], F32R, name=f"vn{p_}{j}{b}") for b in range(B)]
                        for j in range(HPC)]

                # ---------- P1: kv_a on my 512-token shard + rms norm + AllGather
                with ExitStack() as c1:
                    p1 = c1.enter_context(tc.tile_pool(name=f"p1_{p_}", bufs=1))
                    p1ps = c1.enter_context(tc.tile_pool(name=f"p1ps_{p_}", bufs=1, space="PSUM"))
                    ps_lat = [p1ps.tile([128, TPC], F32, name=f"pslat{p_}{m}", tag=f"lat{m}")
                              for m in range(NR)]
                    for k in range(NH):
                        kva_t = p1.tile([128, RANK], F32R, tag="kvat", bufs=3)
                        nc.sync.dma_start(kva_t[:], kvaT_in[k * 128:(k + 1) * 128, :])
                        ht = p1.tile([128, TPC], F32R, tag="hsmy", bufs=6)
                        nc.sync.dma_start(ht[:], hsmy_in[k * 128:(k + 1) * 128, :])
                        for m in range(NR):
                            nc.tensor.matmul(
                                ps_lat[m][:],
                                kva_t[:, m * 128:(m + 1) * 128],
                                ht[:], start=(k == 0), stop=(k == NH - 1))
                    # rms norm over rank (partition axis, 4 chunks)
                    lat_sb = p1.tile([128, NR * TPC], F32)
                    ps_var = p1ps.tile([1, TPC], F32, tag="var")
                    for m in range(NR):
                        nc.any.tensor_copy(lat_sb[:, m * TPC:(m + 1) * TPC], ps_lat[m][:])
                    sq = [p1.tile([128, TPC], F32, name=f"sq{p_}{m}", tag="sq", bufs=2)
                          for m in range(NR)]
                    for m in range(NR):
                        nc.vector.tensor_mul(sq[m][:], lat_sb[:, m * TPC:(m + 1) * TPC],
                                             lat_sb[:, m * TPC:(m + 1) * TPC])
                        nc.tensor.matmul(ps_var[:], ones[:, 0:1].bitcast(F32), sq[m][:],
                                         start=(m == 0), stop=(m == NR - 1))
                    std = p1.tile([1, TPC], F32, tag="std")
                    nc.scalar.activation(std[:], ps_var[:], AF.Sqrt,
                                         bias=eps_t[:], scale=1.0 / RANK)
                    istd = p1.tile([1, TPC], F32, tag="istd")
                    nc.vector.reciprocal(istd[:], std[:])
                    ps_bc = p1ps.tile([128, TPC], F32, tag="bc")
                    nc.tensor.matmul(ps_bc[:], ones[0:1, :].bitcast(F32), istd[:],
                                     start=True, stop=True)
                    latn = p1.tile([128, NR * TPC], F32R)
                    for h in range(2):
                        for m2 in range(2):
                            m = 2 * h + m2
                            nc.vector.tensor_mul(latn[:, m * TPC:(m + 1) * TPC],
                                                 lat_sb[:, m * TPC:(m + 1) * TPC],
                                                 ps_bc[:])
                            nc.sync.dma_start(ag_in[h][m2 * 128:(m2 + 1) * 128, :],
                                              latn[:, m * TPC:(m + 1) * TPC])
                        if sim_mode:
                            for s8 in range(NCORES):
                                nc.sync.dma_start(ag_out[h][s8], ag_in[h][:])
                        else:
                            nc.gpsimd.collective_compute(
                                "AllGather", mybir.AluOpType.bypass,
                                replica_groups=RG,
                                ins=[ag_in[h].opt()], outs=[ag_out[h].opt()])

                # ---------- P2: q_proj for my 2 heads over all 4096 tokens
                with ExitStack() as c2:
                    p2 = c2.enter_context(tc.tile_pool(name=f"p2_{p_}", bufs=1))
                    p2ps = c2.enter_context(tc.tile_pool(name=f"p2ps_{p_}", bufs=1, space="PSUM"))
                    for g in range(4):            # 1024-token groups
                        b, half = g // 2, g % 2
                        psq = [[p2ps.tile([128, 512], F32, name=f"psq{p_}{g}{m}{t2}",
                                          tag="psq", bufs=8)
                                for t2 in range(2)] for m in range(HPC)]
                        for k in range(NH):
                            qw_t = p2.tile([128, HPC * D], F32R, tag="qwt", bufs=4)
                            nc.sync.dma_start(qw_t[:],
                                              qwT_in[k * 128:(k + 1) * 128, :])
                            ht = p2.tile([128, 1024], F32R, tag="hsq", bufs=6)
                            nc.sync.dma_start(
                                ht[:], hsT_in[k * 128:(k + 1) * 128,
                                              g * 1024:(g + 1) * 1024])
                            for m in range(HPC):
                                for t2 in range(2):
                                    nc.tensor.matmul(
                                        psq[m][t2][:],
                                        qw_t[:, m * 128:(m + 1) * 128],
                                        ht[:, t2 * 512:(t2 + 1) * 512],
                                        start=(k == 0), stop=(k == NH - 1))
                        for m in range(HPC):
                            for t2 in range(2):
                                col = half * 1024 + t2 * 512
                                nc.any.tensor_copy(qT[m][b][:, col:col + 512],
                                                   psq[m][t2][:])
                        if half == 1:
                            for j in range(HPC):
                                rope_block(qT[j][b])

                # ---------- P3: kv_b for my 2 heads over all tokens (+ v transpose)
                with ExitStack() as c3:
                    p3 = c3.enter_context(tc.tile_pool(name=f"p3_{p_}", bufs=1))
                    p3ps = c3.enter_context(tc.tile_pool(name=f"p3ps_{p_}", bufs=1, space="PSUM"))
                    kvbT_sb = p3.tile([128, NR * HPC * 2 * D], F32R)
                    nc.sync.dma_start(
                        kvbT_sb[:].rearrange("p (r m) -> p r m", r=NR),
                        kvbT_in.rearrange("(r p) m -> p r m", p=128))
                    for tc8 in range(NCORES):     # 512-token chunks (AG layout)
                        b, loc = tc8 // 4, (tc8 % 4) * 512
                        lt = [p3.tile([128, 2 * 512], F32R, tag=f"lt{h}", bufs=4,
                                      name=f"lth{h}") for h in range(2)]
                        for h in range(2):
                            nc.sync.dma_start(
                                lt[h][:].rearrange("p (r t) -> p r t", r=2),
                                ag_out[h][tc8].rearrange("(r p) t -> p r t", p=128))
                        for m in range(2 * HPC):  # k0,v0,k1,v1
                            j, is_v = m // 2, m % 2
                            ps = p3ps.tile([128, 512], F32, tag="kv", bufs=4)
                            for r in range(NR):
                                nc.tensor.matmul(
                                    ps[:],
                                    kvbT_sb[:, r * HPC * 2 * D + m * 128:
                                            r * HPC * 2 * D + (m + 1) * 128],
                                    lt[r // 2][:, (r % 2) * 512:(r % 2 + 1) * 512],
                                    start=(r == 0), stop=(r == NR - 1))
                            if not is_v:
                                nc.any.tensor_copy(kT[j][b][:, loc:loc + 512], ps[:])
                            else:
                                vt = p3.tile([128, 512], F32R, tag="vt", bufs=2)
                                nc.any.tensor_copy(vt[:], ps[:])
                                for q4 in range(4):
                                    pst = p3ps.tile([128, 128], F32R, tag="pst", bufs=2)
                                    nc.tensor.transpose(
                                        pst[:], vt[:, q4 * 128:(q4 + 1) * 128], ident[:])
                                    nc.any.tensor_copy(
                                        vnat[j][b][:, loc + q4 * 128: loc + (q4 + 1) * 128],
                                        pst[:])
                        if tc8 % 4 == 3:
                            for j in range(HPC):
                                rope_block(kT[j][b])

                # ---------- P5: attention per (batch, head), scoresT layout
                with ExitStack() as c5:
                    p5 = c5.enter_context(tc.tile_pool(name=f"p5_{p_}", bufs=1))
                    p5ps = c5.enter_context(tc.tile_pool(name=f"p5ps_{p_}", bufs=1, space="PSUM"))
                    NKT = S // 128   # 16 k-chunks per batch
                    for b in range(B):
                        for j in range(HPC):
                            qt, kt, vn = qT[j][b], kT[j][b], vnat[j][b]
                            for qc in range(4):
                                qs = qt[:, qc * 512:(qc + 1) * 512]
                                ps_av = p5ps.tile([128, 512], F32, tag="av", bufs=2)
                                ps_den = p5ps.tile([128, 512], F32, tag="den", bufs=2)
                                for kp in range(NKT // 2):
                                    ps_s = p5ps.tile([128, 1024], F32, tag="s", bufs=2)
                                    e = p5.tile([128, 1024], F32R, tag="e", bufs=6)
                                    for h2 in range(2):
                                        k16 = 2 * kp + h2
                                        nc.tensor.matmul(
                                            ps_s[:, h2 * 512:(h2 + 1) * 512],
                                            kt[:, k16 * 128:(k16 + 1) * 128], qs,
                                            start=True, stop=True)
                                    nc.scalar.activation(e[:], ps_s[:], AF.Exp)
                                    for h2 in range(2):
                                        k16 = 2 * kp + h2
                                        es = e[:, h2 * 512:(h2 + 1) * 512]
                                        nc.tensor.matmul(
                                            ps_av[:], vn[:, k16 * 128:(k16 + 1) * 128], es,
                                            start=(k16 == 0), stop=(k16 == NKT - 1))
                                        nc.tensor.matmul(
                                            ps_den[0:1, :], ones[:, 0:1], es,
                                            start=(k16 == 0), stop=(k16 == NKT - 1))
                                den_sb = p5.tile([1, 512], F32R, tag="densb", bufs=3)
                                nc.vector.tensor_copy(den_sb[:], ps_den[0:1, :])
                                # broadcast back into the same (now free) den bank
                                nc.tensor.matmul(ps_den[:], ones[0:1, :], den_sb[:],
                                                 start=True, stop=True)
                                rec = p5.tile([128, 512], F32, tag="rec", bufs=3)
                                nc.vector.reciprocal(rec[:], ps_den[:])
                                ao_t = p5.tile([128, 512], F32R, tag="aot", bufs=4)
                                nc.vector.tensor_mul(ao_t[:], ps_av[:], rec[:])
                                for h2a in range(2):
                                    s8 = 2 * qc + h2a
                                    nc.sync.dma_start(
                                        a2a_in[b][s8, j * D:(j + 1) * D, :],
                                        ao_t[:, h2a * SPC:(h2a + 1) * SPC])
                        # AllToAll for this batch as soon as both heads are done
                        if sim_mode:
                            nc.sync.dma_start(a2a_out[b][:], a2a_in[b][:])
                        else:
                            nc.gpsimd.collective_compute(
                                "AllToAll", mybir.AluOpType.bypass, replica_groups=RG,
                                ins=[a2a_in[b].opt()], outs=[a2a_out[b].opt()])
                        for k16 in range(NH):
                            i, halfk = k16 // 2, k16 % 2
                            nc.sync.dma_start(
                                af[:, k16 * 2 * SPC + b * SPC:
                                   k16 * 2 * SPC + (b + 1) * SPC],
                                a2a_out[b][i, halfk * 128:(halfk + 1) * 128, :])

            # ---------- P7: o_proj on my 512 tokens (256 per batch)
            with ExitStack() as c7:
                p7 = c7.enter_context(tc.tile_pool(name=f"p7_{p_}", bufs=1))
                p7ps = c7.enter_context(tc.tile_pool(name=f"p7ps_{p_}", bufs=1, space="PSUM"))
                for om in range(NH):
                    wt = p7.tile([128, HID], F32R, tag="ow", bufs=4)
                    nc.sync.dma_start(wt[:], owt_in[om])
                    ps_o = p7ps.tile([128, 2 * SPC], F32, tag="o", bufs=4)
                    for k16 in range(NH):
                        nc.tensor.matmul(
                            ps_o[:], wt[:, k16 * 128:(k16 + 1) * 128],
                            af[:, k16 * 2 * SPC:(k16 + 1) * 2 * SPC],
                            start=(k16 == 0), stop=(k16 == NH - 1))
                    o_sb = p7.tile([128, 2 * SPC], F32, tag="osb", bufs=3)
                    nc.any.tensor_copy(o_sb[:], ps_o[:])
                    nc.sync.dma_start(outTs[p_][om * 128:(om + 1) * 128, :], o_sb[:])
            ctx_pass.close()

    nc.compile()
    return nc


def build_in_maps(hidden_states, q_w, kv_a_w, kv_b_w, o_w, kv_norm_w):
    hs = np.ascontiguousarray(np.asarray(hidden_states, dtype=np.float32))
    q_w = np.asarray(q_w, dtype=np.float32)
    kv_a_w = np.asarray(kv_a_w, dtype=np.float32)
    kv_b_w = np.asarray(kv_b_w, dtype=np.float32)
    o_w = np.asarray(o_w, dtype=np.float32)
    kv_norm_w = np.asarray(kv_norm_w, dtype=np.float32)

    hsT = tf32_round(np.ascontiguousarray(hs.reshape(T, HID).T))      # [HID, T]
    kvaT = tf32_round(np.ascontiguousarray(kv_a_w[ROPE:, :].T))       # [HID, RANK]
    scale = D ** -0.5
    cosT, sinT = _yarn_cos_sin()
    sinsh = np.concatenate([sinT[32:64], -sinT[0:32]], axis=0)
    ident = np.eye(128, dtype=np.float32)
    ones = np.ones((128, 128), dtype=np.float32)
    # owt[om, p, k*128+m] = o_w[om*128+m, k*128+p]
    owt = tf32_round(np.ascontiguousarray(
        o_w.reshape(16, 128, 16, 128).transpose(0, 3, 2, 1).reshape(16, 128, HID)))

    kvb = (kv_b_w * kv_norm_w[None, :]).reshape(H, 2, D, RANK)

    in_maps = []
    for c in range(NCORES):
        qwT = tf32_round(np.ascontiguousarray(
            (q_w[c * HPC * D:(c + 1) * HPC * D] * scale).T))           # [HID, 256]
        # kvbT rows order per core: k0,v0,k1,v1 each 128 wide
        blk = kvb[c * HPC:(c + 1) * HPC]                               # [2,2,128,RANK]
        kvbT = tf32_round(np.ascontiguousarray(
            blk.reshape(HPC * 2 * D, RANK).T))                         # [RANK, 512]
        hsmy = tf32_round(np.ascontiguousarray(
            hsT[:, c * TPC:(c + 1) * TPC]))
        in_maps.append({
            "hsT": hsT, "hsmy": hsmy, "qwT": qwT, "kvaT": kvaT,
            "kvbT": kvbT, "owt": owt, "cos": cosT, "sinsh": sinsh,
            "ident": ident, "ones": ones,
        })
    return in_maps


def assemble_output(results):
    out = np.empty((B, S, HID), dtype=np.float32)
    for c in range(NCORES):
        r = results[c]["outT"]                 # [HID, 2*SPC]
        out[0, c * SPC:(c + 1) * SPC, :] = r[:, 0:SPC].T
        out[1, c * SPC:(c + 1) * SPC, :] = r[:, SPC:2 * SPC].T
    return out


def kernel(hidden_states, q_w, kv_a_w, kv_b_w, o_w, kv_norm_w):
    from concourse import bass_utils

    if "nc" not in _CACHE:
        _CACHE["nc"] = build_nc()
    nc = _CACHE["nc"]
    in_maps = build_in_maps(hidden_states, q_w, kv_a_w, kv_b_w, o_w, kv_norm_w)
    res = bass_utils.run_bass_kernel_spmd(
        nc, in_maps, core_ids=list(range(NCORES)), trace=False)
    return assemble_output(res.results)



# revision 15
# speedup vs baseline: 1.1750x; 1.1750x over previous
"""MultiHeadLatentAttention on 8 Trainium2 NeuronCores (Bass/Tile, SPMD).

Sharding (tensor parallel over heads, per the hint, plus two refinements):
  - 16 heads / 8 cores = 2 heads per core: q_proj + kv_b_proj output dims and
    o_proj input dim sharded by head.
  - kv_a_proj + rms-norm are token-sharded (512 tokens/core) with an
    AllGather of the normalized latent (small: 1 MB/core) instead of
    replicating the 9.7 GFLOP kv_a matmul on every core.
  - Instead of an AllReduce of full [B,S,H] partial o_proj outputs (33 MB,
    ~380 us), an AllToAll of the attention outputs (2 MB bf16) token-shards
    the o_proj: each core computes the full o_proj for 512 tokens and outputs
    exactly its token slice. Host-side unshard is a pure concat/transpose.

Precision: q/k path is fp32r (TF32-grid inputs rounded on host, fp32
accumulate, 1 cycle/row on the PE) so exp(scores) stays accurate. The value
side runs bf16 (v tiles, exp outputs, o_proj weights + attention outputs) --
bf16 matmuls are also 1 cycle/row, DVE element-wise ops run 2x on bf16, and
HBM/collective traffic halves; measured end-to-end rel err ~3e-3 (gate 2e-2).

Softmax denominators: summing exp chunks with ones-matmuls on the PE costs
as much PE time as the AV matmul itself (~90 us/pass measured), so the
per-128-chunk exp sums are accumulated on the otherwise-idle Pool + Vector
engines (bf16 chains), leaving one [1,512] ones-matmul + reciprocal per
512-query block. The den matmul for a block is deferred by one block so the
PE never waits on the chains.

Schedule: P1 kv_a+AllGather -> P2 q_proj (q weights SBUF-resident) ->
P3 kv_b (+v transpose to [tok,d] bf16) -> P5 attention (per batch: scoresT
[ktok,qtok] k-stationary matmuls, exp on Act, AV accumulate, AllToAll per
batch overlapped with the next batch / o_proj weight prefetch) -> P7 o_proj
(bf16 weights prefetched past the AllToAll-gated af copies so they stream
during the collective).
"""

import math
from contextlib import ExitStack

import numpy as np

B, S = 2, 2048
T = B * S                     # 4096 flattened tokens
HID = 2048
H, D = 16, 128
RANK, ROPE = 512, 64
MAX_POS, ORIG_POS = 131072, 8192
BASE = 500000.0
BETA_FAST, BETA_SLOW = 32.0, 1.0
EPS = 1e-6
NCORES = 8
HPC = H // NCORES             # 2 heads per core
TPC = T // NCORES             # 512 tokens per core (kv_a shard)
SPC = S // NCORES             # 256 tokens per (core, batch) after AllToAll

_CACHE: dict = {}


def tf32_round(x: np.ndarray) -> np.ndarray:
    u = np.ascontiguousarray(x, dtype=np.float32).view(np.uint32).copy()
    add = ((u >> 13) & 1) + 0xFFF
    u = (u + add) & np.uint32(0xFFFFE000)
    return u.view(np.float32)


def _yarn_cos_sin():
    """cos/sin tables matching reference.py's yarn_cos_sin (mscale folded)."""
    scaling = MAX_POS / ORIG_POS
    pos_freqs = BASE ** (np.arange(0, ROPE, 2, dtype=np.float64) / ROPE)
    extrap = 1.0 / pos_freqs
    interp = 1.0 / (scaling * pos_freqs)
    low = max(math.floor(ROPE * math.log(ORIG_POS / (BETA_FAST * 2 * math.pi))
                         / (2 * math.log(BASE))), 0)
    high = min(math.ceil(ROPE * math.log(ORIG_POS / (BETA_SLOW * 2 * math.pi))
                         / (2 * math.log(BASE))), ROPE - 1)
    i = np.arange(ROPE // 2, dtype=np.float64)
    smooth = np.clip((i - low) / max(high - low, 1), 0.0, 1.0)
    inv_freq = ((1.0 - smooth) * interp + smooth * extrap).astype(np.float32)
    pos = np.arange(S, dtype=np.float32)
    freqs = pos[:, None] * inv_freq[None, :]              # [S, 32]
    emb = np.concatenate([freqs, freqs], axis=-1)         # [S, 64]
    mscale = 0.1 * math.log(scaling) + 1.0
    cos = (np.cos(emb) * mscale).astype(np.float32)
    sin = (np.sin(emb) * mscale).astype(np.float32)
    return cos.T.copy(), sin.T.copy()                     # [64, S] each


def build_nc(passes=1, sim_mode=False, den=1, p7=1, coll=1, npre=0):
    """Build + compile the (single, SPMD) Bass program for all 8 cores.

    den/p7/coll are timing-ablation flags (1 = normal). den=0 skips the
    softmax-denominator matmuls (wrong output), p7=0 skips o_proj compute,
    coll=0 replaces collectives with local DMA copies.
    """
    p7f = p7
    import concourse.tile as tile
    import concourse.mybir as mybir
    from concourse import bacc

    F32 = mybir.dt.float32
    F32R = mybir.dt.float32r
    BF16 = mybir.dt.bfloat16
    AF = mybir.ActivationFunctionType
    RG = [list(range(NCORES))]

    nc = bacc.Bacc("TRN2", target_bir_lowering=False, debug=False,
                   num_devices=1 if sim_mode else NCORES)

    # ---- kernel I/O ----
    hsT_in = nc.dram_tensor("hsT", [HID, T], F32R, kind="ExternalInput").ap()
    hsmy_in = nc.dram_tensor("hsmy", [HID, TPC], F32R, kind="ExternalInput").ap()
    qwT_in = nc.dram_tensor("qwT", [HID, HPC * D], F32R, kind="ExternalInput").ap()
    kvaT_in = nc.dram_tensor("kvaT", [HID, RANK], F32R, kind="ExternalInput").ap()
    kvbT_in = nc.dram_tensor("kvbT", [RANK, HPC * 2 * D], F32R, kind="ExternalInput").ap()
    owt_in = nc.dram_tensor("owt", [16, 128, HID], BF16, kind="ExternalInput").ap()
    cos_in = nc.dram_tensor("cos", [ROPE, S], F32, kind="ExternalInput").ap()
    sinsh_in = nc.dram_tensor("sinsh", [ROPE, S], F32, kind="ExternalInput").ap()
    ident_in = nc.dram_tensor("ident", [128, 128], F32R, kind="ExternalInput").ap()
    ones_in = nc.dram_tensor("ones", [128, 128], F32R, kind="ExternalInput").ap()
    outTs = [nc.dram_tensor(f"outT{p}" if p else "outT", [HID, 2 * SPC], F32,
                            kind="ExternalOutput").ap() for p in range(passes)]

    NH = HID // 128   # 16 hid chunks
    NR = RANK // 128  # 4 rank chunks

    with tile.TileContext(nc) as tc, ExitStack() as ctx0:
        const = ctx0.enter_context(tc.tile_pool(name="const", bufs=1))
        dram = ctx0.enter_context(tc.tile_pool(name="dram", bufs=1, space="DRAM"))

        ident = const.tile([128, 128], F32R)
        ones = const.tile([128, 128], F32R)
        ones_bf = const.tile([128, 128], BF16)
        cosb = const.tile([ROPE, S], F32)
        sinsh = const.tile([ROPE, S], F32)
        eps_t = const.tile([1, 1], F32)
        nc.sync.dma_start(ident[:], ident_in[:])
        nc.sync.dma_start(ones[:], ones_in[:])
        nc.vector.tensor_copy(ones_bf[:], ones[:])
        nc.sync.dma_start(cosb[:], cos_in[:])
        nc.sync.dma_start(sinsh[:], sinsh_in[:])
        nc.vector.memset(eps_t[:], EPS)

        for p_ in range(passes):
            # collective bounce buffers
            ag_in = [dram.tile([RANK // 2, TPC], F32R, name=f"agin{p_}{h}")
                     for h in range(2)]
            ag_out = [dram.tile([NCORES, RANK // 2, TPC], F32R,
                                addr_space="Local" if (sim_mode or not coll)
                                else "Shared",
                                name=f"agout{p_}{h}") for h in range(2)]
            a2a_in = [dram.tile([NCORES, HPC * D, SPC], BF16, name=f"a2ain{p_}{b}")
                      for b in range(B)]
            a2a_out = [dram.tile([NCORES, HPC * D, SPC], BF16, name=f"a2aout{p_}{b}")
                       for b in range(B)]

            ctx_pass = ExitStack()
            afp = ctx_pass.enter_context(tc.tile_pool(name=f"afp_{p_}", bufs=1))
            af = afp.tile([128, NH * 2 * SPC], BF16, name=f"af{p_}")
            with ExitStack() as ctx_big:
                big = ctx_big.enter_context(tc.tile_pool(name=f"big_{p_}", bufs=1))
                rope_pool = ctx_big.enter_context(
                    tc.tile_pool(name=f"rope_{p_}", bufs=1))

                def rope_block(X):
                    tmp = rope_pool.tile([ROPE, S], F32, tag="rtmp", bufs=1,
                                         name="rtmp")
                    m2 = rope_pool.tile([ROPE, S], F32, tag="rm2", bufs=1,
                                        name="rm2")
                    nc.vector.tensor_mul(tmp[:], X[0:ROPE], cosb[:])
                    nc.vector.tensor_mul(m2[0:32], X[32:64], sinsh[32:64])
                    nc.vector.tensor_mul(m2[32:64], X[0:32], sinsh[0:32])
                    nc.vector.tensor_add(X[0:ROPE], tmp[:], m2[:])

                # per (head j, batch b) tiles, [128, S] each
                qT = [[big.tile([128, S], F32R, name=f"qT{p_}{j}{b}") for b in range(B)]
                      for j in range(HPC)]
                kT = [[big.tile([128, S], F32R, name=f"kT{p_}{j}{b}") for b in range(B)]
                      for j in range(HPC)]
                vnat = [[big.tile([128, S], BF16, name=f"vn{p_}{j}{b}") for b in range(B)]
                        for j in range(HPC)]

                # ---------- P1: kv_a on my 512-token shard + rms norm + AllGather
                with ExitStack() as c1:
                    p1 = c1.enter_context(tc.tile_pool(name=f"p1_{p_}", bufs=1))
                    p1ps = c1.enter_context(tc.tile_pool(name=f"p1ps_{p_}", bufs=1, space="PSUM"))
                    ps_lat = [p1ps.tile([128, TPC], F32, name=f"pslat{p_}{m}", tag=f"lat{m}")
                              for m in range(NR)]
                    for k in range(NH):
                        kva_t = p1.tile([128, RANK], F32R, tag="kvat", bufs=3)
                        nc.sync.dma_start(kva_t[:], kvaT_in[k * 128:(k + 1) * 128, :])
                        ht = p1.tile([128, TPC], F32R, tag="hsmy", bufs=6)
                        nc.sync.dma_start(ht[:], hsmy_in[k * 128:(k + 1) * 128, :])
                        for m in range(NR):
                            nc.tensor.matmul(
                                ps_lat[m][:],
                                kva_t[:, m * 128:(m + 1) * 128],
                                ht[:], start=(k == 0), stop=(k == NH - 1))
                    # rms norm over rank (partition axis, 4 chunks)
                    lat_sb = p1.tile([128, NR * TPC], F32)
                    ps_var = p1ps.tile([1, TPC], F32, tag="var")
                    for m in range(NR):
                        nc.any.tensor_copy(lat_sb[:, m * TPC:(m + 1) * TPC], ps_lat[m][:])
                    sq = [p1.tile([128, TPC], F32, name=f"sq{p_}{m}", tag="sq", bufs=2)
                          for m in range(NR)]
                    for m in range(NR):
                        nc.vector.tensor_mul(sq[m][:], lat_sb[:, m * TPC:(m + 1) * TPC],
                                             lat_sb[:, m * TPC:(m + 1) * TPC])
                        nc.tensor.matmul(ps_var[:], ones[:, 0:1].bitcast(F32), sq[m][:],
                                         start=(m == 0), stop=(m == NR - 1))
                    std = p1.tile([1, TPC], F32, tag="std")
                    nc.scalar.activation(std[:], ps_var[:], AF.Sqrt,
                                         bias=eps_t[:], scale=1.0 / RANK)
                    istd = p1.tile([1, TPC], F32, tag="istd")
                    nc.vector.reciprocal(istd[:], std[:])
                    ps_bc = p1ps.tile([128, TPC], F32, tag="bc")
                    nc.tensor.matmul(ps_bc[:], ones[0:1, :].bitcast(F32), istd[:],
                                     start=True, stop=True)
                    latn = p1.tile([128, NR * TPC], F32R)
                    for h in range(2):
                        for m2 in range(2):
                            m = 2 * h + m2
                            nc.vector.tensor_mul(latn[:, m * TPC:(m + 1) * TPC],
                                                 lat_sb[:, m * TPC:(m + 1) * TPC],
                                                 ps_bc[:])
                            nc.sync.dma_start(ag_in[h][m2 * 128:(m2 + 1) * 128, :],
                                              latn[:, m * TPC:(m + 1) * TPC])
                        if sim_mode or not coll:
                            for s8 in range(NCORES):
                                nc.sync.dma_start(ag_out[h][s8], ag_in[h][:])
                        else:
                            nc.gpsimd.collective_compute(
                                "AllGather", mybir.AluOpType.bypass,
                                replica_groups=RG,
                                ins=[ag_in[h].opt()], outs=[ag_out[h].opt()])

                # ---------- P2: q_proj for my 2 heads over all 4096 tokens
                with ExitStack() as c2:
                    p2 = c2.enter_context(tc.tile_pool(name=f"p2_{p_}", bufs=1))
                    p2ps = c2.enter_context(tc.tile_pool(name=f"p2ps_{p_}", bufs=1, space="PSUM"))
                    qw_sb = p2.tile([128, NH * HPC * D], F32R)  # resident, 2 MB
                    nc.sync.dma_start(
                        qw_sb[:].rearrange("p (k m) -> p k m", k=NH),
                        qwT_in.rearrange("(k p) m -> p k m", p=128))
                    for g in range(4):            # 1024-token groups
                        b, half = g // 2, g % 2
                        psq = [[p2ps.tile([128, 512], F32, name=f"psq{p_}{g}{m}{t2}",
                                          tag="psq", bufs=8)
                                for t2 in range(2)] for m in range(HPC)]
                        for k in range(NH):
                            ht = p2.tile([128, 1024], F32R, tag="hsq", bufs=6)
                            nc.sync.dma_start(
                                ht[:], hsT_in[k * 128:(k + 1) * 128,
                                              g * 1024:(g + 1) * 1024])
                            for m in range(HPC):
                                for t2 in range(2):
                                    nc.tensor.matmul(
                                        psq[m][t2][:],
                                        qw_sb[:, k * HPC * D + m * 128:
                                              k * HPC * D + (m + 1) * 128],
                                        ht[:, t2 * 512:(t2 + 1) * 512],
                                        start=(k == 0), stop=(k == NH - 1))
                        for m in range(HPC):
                            for t2 in range(2):
                                col = half * 1024 + t2 * 512
                                nc.any.tensor_copy(qT[m][b][:, col:col + 512],
                                                   psq[m][t2][:])
                        if half == 1:
                            for j in range(HPC):
                                rope_block(qT[j][b])

                # ---------- P3: kv_b for my 2 heads over all tokens (+ v transpose)
                with ExitStack() as c3:
                    p3 = c3.enter_context(tc.tile_pool(name=f"p3_{p_}", bufs=1))
                    p3ps = c3.enter_context(tc.tile_pool(name=f"p3ps_{p_}", bufs=1, space="PSUM"))
                    kvbT_sb = p3.tile([128, NR * HPC * 2 * D], F32R)
                    nc.sync.dma_start(
                        kvbT_sb[:].rearrange("p (r m) -> p r m", r=NR),
                        kvbT_in.rearrange("(r p) m -> p r m", p=128))
                    for tc8 in range(NCORES):     # 512-token chunks (AG layout)
                        b, loc = tc8 // 4, (tc8 % 4) * 512
                        lt = [p3.tile([128, 2 * 512], F32R, tag=f"lt{h}", bufs=4,
                                      name=f"lth{h}") for h in range(2)]
                        for h in range(2):
                            nc.sync.dma_start(
                                lt[h][:].rearrange("p (r t) -> p r t", r=2),
                                ag_out[h][tc8].rearrange("(r p) t -> p r t", p=128))
                        for m in range(2 * HPC):  # k0,v0,k1,v1
                            j, is_v = m // 2, m % 2
                            ps = p3ps.tile([128, 512], F32, tag="kv", bufs=4)
                            for r in range(NR):
                                nc.tensor.matmul(
                                    ps[:],
                                    kvbT_sb[:, r * HPC * 2 * D + m * 128:
                                            r * HPC * 2 * D + (m + 1) * 128],
                                    lt[r // 2][:, (r % 2) * 512:(r % 2 + 1) * 512],
                                    start=(r == 0), stop=(r == NR - 1))
                            if not is_v:
                                nc.any.tensor_copy(kT[j][b][:, loc:loc + 512], ps[:])
                            else:
                                vt = p3.tile([128, 512], F32R, tag="vt", bufs=2)
                                nc.any.tensor_copy(vt[:], ps[:])
                                for q4 in range(4):
                                    pst = p3ps.tile([128, 128], F32R, tag="pst", bufs=2)
                                    nc.tensor.transpose(
                                        pst[:], vt[:, q4 * 128:(q4 + 1) * 128], ident[:])
                                    nc.any.tensor_copy(
                                        vnat[j][b][:, loc + q4 * 128: loc + (q4 + 1) * 128],
                                        pst[:])
                        if tc8 % 4 == 3:
                            for j in range(HPC):
                                rope_block(kT[j][b])

                def emit_p7(bb, pool, pspool, tagsuf, psbufs, oms=None):
                    """o_proj for batch bb's token slice (om-chunked)."""
                    for om in (range(NH) if oms is None else oms):
                        wt = pool.tile([128, HID], BF16, tag=f"ow{tagsuf}",
                                       bufs=4, name=f"wt{tagsuf}")
                        nc.sync.dma_start(wt[:], owt_in[om])
                        ps_o = pspool.tile([128, SPC], F32, tag=f"o7{tagsuf}",
                                           bufs=psbufs, name=f"pso7{tagsuf}")
                        for k16 in range(NH):
                            nc.tensor.matmul(
                                ps_o[:], wt[:, k16 * 128:(k16 + 1) * 128],
                                af[:, k16 * 2 * SPC + bb * SPC:
                                   k16 * 2 * SPC + (bb + 1) * SPC],
                                start=(k16 == 0), stop=(k16 == NH - 1))
                        o_sb = pool.tile([128, SPC], F32, tag=f"osb{tagsuf}",
                                         bufs=3, name=f"osb{tagsuf}")
                        nc.any.tensor_copy(o_sb[:], ps_o[:])
                        nc.sync.dma_start(
                            outTs[p_][om * 128:(om + 1) * 128,
                                      bb * SPC:(bb + 1) * SPC], o_sb[:])

                # ---------- P5: attention per (batch, head), scoresT layout
                with ExitStack() as c5:
                    p5 = c5.enter_context(tc.tile_pool(name=f"p5_{p_}", bufs=1))
                    p5ps = c5.enter_context(tc.tile_pool(name=f"p5ps_{p_}", bufs=1, space="PSUM"))
                    NKT = S // 128   # 16 k-chunks per batch

                    def finalize(pend):
                        """Normalize + ship qc whose exp-sum chains are done.

                        Deferred by ~one qc so the PE's den matmul never
                        waits on the DVE/Pool accumulation chains.
                        """
                        ps_av, accD, b, j, qc = pend
                        ps_dn = p5ps.tile([128, 512], F32, tag="dn", bufs=1)
                        nc.tensor.matmul(ps_dn[0:1, :], ones_bf[:, 0:1], accD[:],
                                         start=True, stop=True)
                        den_sb = p5.tile([1, 512], F32R, tag="densb", bufs=2)
                        nc.vector.tensor_copy(den_sb[:], ps_dn[0:1, :])
                        # broadcast into the full (same) dn bank
                        nc.tensor.matmul(ps_dn[:], ones[0:1, :], den_sb[:],
                                         start=True, stop=True)
                        rec = p5.tile([128, 512], F32, tag="rec", bufs=2)
                        nc.vector.reciprocal(rec[:], ps_dn[:])
                        ao_t = p5.tile([128, 512], BF16, tag="aot", bufs=3)
                        nc.vector.tensor_mul(ao_t[:], ps_av[:], rec[:])
                        for h2a in range(2):
                            s8 = 2 * qc + h2a
                            nc.sync.dma_start(
                                a2a_in[b][s8, j * D:(j + 1) * D, :],
                                ao_t[:, h2a * SPC:(h2a + 1) * SPC])

                    def emit_a2a(b, af_copies=True):
                        if sim_mode or not coll:
                            nc.sync.dma_start(a2a_out[b][:], a2a_in[b][:])
                        else:
                            nc.gpsimd.collective_compute(
                                "AllToAll", mybir.AluOpType.bypass,
                                replica_groups=RG,
                                ins=[a2a_in[b].opt()], outs=[a2a_out[b].opt()])
                        if af_copies:
                            emit_af(b)

                    def emit_af(b):
                        for k16 in range(NH):
                            i, halfk = k16 // 2, k16 % 2
                            nc.sync.dma_start(
                                af[:, k16 * 2 * SPC + b * SPC:
                                   k16 * 2 * SPC + (b + 1) * SPC],
                                a2a_out[b][i, halfk * 128:(halfk + 1) * 128, :])

                    pend = None
                    a2a_due = None
                    af_deferred = None
                    for b in range(B):
                        for j in range(HPC):
                            qt, kt, vn = qT[j][b], kT[j][b], vnat[j][b]
                            for qc in range(4):
                                qs = qt[:, qc * 512:(qc + 1) * 512]
                                ps_av = p5ps.tile([128, 512], F32, tag="av", bufs=2)
                                if den:
                                    accD = p5.tile([128, 512], BF16, tag="accD", bufs=2)
                                    accP = p5.tile([128, 512], BF16, tag="accP", bufs=2)
                                for kp in range(NKT // 2):
                                    ps_s = p5ps.tile([128, 1024], F32, tag="s", bufs=2)
                                    e = p5.tile([128, 1024], BF16, tag="e", bufs=6)
                                    for h2 in range(2):
                                        k16 = 2 * kp + h2
                                        nc.tensor.matmul(
                                            ps_s[:, h2 * 512:(h2 + 1) * 512],
                                            kt[:, k16 * 128:(k16 + 1) * 128], qs,
                                            start=True, stop=True)
                                    nc.scalar.activation(e[:], ps_s[:], AF.Exp)
                                    for h2 in range(2):
                                        k16 = 2 * kp + h2
                                        es = e[:, h2 * 512:(h2 + 1) * 512]
                                        nc.tensor.matmul(
                                            ps_av[:], vn[:, k16 * 128:(k16 + 1) * 128], es,
                                            start=(k16 == 0), stop=(k16 == NKT - 1))
                                    if den:
                                        # bf16 exp-sum chains (DVE runs 2x on
                                        # bf16): Pool takes kp 0-1, DVE the
                                        # rest + merge
                                        c0 = e[:, 0:512]
                                        c1 = e[:, 512:1024]
                                        if kp == 0:
                                            nc.gpsimd.tensor_add(accP[:], c0, c1)
                                        elif kp == 1:
                                            nc.gpsimd.tensor_add(accP[:], accP[:], c0)
                                            nc.gpsimd.tensor_add(accP[:], accP[:], c1)
                                        elif kp == 2:
                                            nc.vector.tensor_add(accD[:], c0, c1)
                                        else:
                                            nc.vector.tensor_add(accD[:], accD[:], c0)
                                            nc.vector.tensor_add(accD[:], accD[:], c1)
                                    if kp == 2:
                                        if pend is not None:
                                            finalize(pend)
                                            pend = None
                                        if a2a_due is not None:
                                            emit_a2a(a2a_due)
                                            a2a_due = None
                                if den:
                                    nc.vector.tensor_add(accD[:], accD[:], accP[:])
                                    pend = (ps_av, accD, b, j, qc)
                                else:
                                    ao_t = p5.tile([128, 512], BF16, tag="aot", bufs=4)
                                    nc.vector.tensor_scalar_mul(ao_t[:], ps_av[:], 0.01)
                                    for h2a in range(2):
                                        s8 = 2 * qc + h2a
                                        nc.sync.dma_start(
                                            a2a_in[b][s8, j * D:(j + 1) * D, :],
                                            ao_t[:, h2a * SPC:(h2a + 1) * SPC])
                        # all of batch b's ao_t exist once the deferred qc is
                        # flushed; if a deferral is pending, let the next
                        # group's kp==2 slot flush it, then fire the AllToAll
                        if pend is None:
                            emit_a2a(b, af_copies=(b == 0))
                            if b == 1:
                                af_deferred = b
                        else:
                            a2a_due = b
                    if pend is not None:
                        finalize(pend)
                        pend = None
                    if a2a_due is not None:
                        emit_a2a(a2a_due, af_copies=False)
                        af_deferred = a2a_due
                        a2a_due = None

            # ---------- P7: o_proj tail (weights prefetched past the
            # AllToAll-gated af copies so they stream during the collective)
            with ExitStack() as c7:
                p7 = c7.enter_context(tc.tile_pool(name=f"p7_{p_}", bufs=1))
                p7ps = c7.enter_context(tc.tile_pool(name=f"p7ps_{p_}", bufs=1, space="PSUM"))
                if p7f:
                    NPRE = npre
                    if af_deferred is None:
                        NPRE = 0
                    wts = []
                    for om in range(NPRE):
                        wt = p7.tile([128, HID], BF16, tag="ow", bufs=12,
                                     name="wt")
                        nc.sync.dma_start(wt[:], owt_in[om])
                        wts.append(wt)
                    if af_deferred is not None:
                        emit_af(af_deferred)
                    for om in range(NH):
                        if om < NPRE:
                            wt = wts[om]
                        else:
                            wt = p7.tile([128, HID], BF16, tag="ow",
                                         bufs=12, name="wt")
                            nc.sync.dma_start(wt[:], owt_in[om])
                        ps_o = p7ps.tile([128, 2 * SPC], F32, tag="o", bufs=4)
                        for k16 in range(NH):
                            nc.tensor.matmul(
                                ps_o[:], wt[:, k16 * 128:(k16 + 1) * 128],
                                af[:, k16 * 2 * SPC:(k16 + 1) * 2 * SPC],
                                start=(k16 == 0), stop=(k16 == NH - 1))
                        o_sb = p7.tile([128, 2 * SPC], F32, tag="osb", bufs=3)
                        nc.any.tensor_copy(o_sb[:], ps_o[:])
                        nc.sync.dma_start(
                            outTs[p_][om * 128:(om + 1) * 128, :], o_sb[:])
                else:
                    for om in range(NH):
                        o_sb = p7.tile([128, 2 * SPC], F32, tag="osb", bufs=3)
                        nc.vector.tensor_copy(o_sb[:], af[:, 0:2 * SPC])
                        nc.sync.dma_start(
                            outTs[p_][om * 128:(om + 1) * 128, :], o_sb[:])
            ctx_pass.close()

    nc.compile()
    return nc


def build_in_maps(hidden_states, q_w, kv_a_w, kv_b_w, o_w, kv_norm_w):
    hs = np.ascontiguousarray(np.asarray(hidden_states, dtype=np.float32))
    q_w = np.asarray(q_w, dtype=np.float32)
    kv_a_w = np.asarray(kv_a_w, dtype=np.float32)
    kv_b_w = np.asarray(kv_b_w, dtype=np.float32)
    o_w = np.asarray(o_w, dtype=np.float32)
    kv_norm_w = np.asarray(kv_norm_w, dtype=np.float32)

    hsT = tf32_round(np.ascontiguousarray(hs.reshape(T, HID).T))      # [HID, T]
    kvaT = tf32_round(np.ascontiguousarray(kv_a_w[ROPE:, :].T))       # [HID, RANK]
    scale = D ** -0.5
    cosT, sinT = _yarn_cos_sin()
    sinsh = np.concatenate([sinT[32:64], -sinT[0:32]], axis=0)
    ident = np.eye(128, dtype=np.float32)
    ones = np.ones((128, 128), dtype=np.float32)
    import ml_dtypes
    # owt[om, p, k*128+m] = o_w[om*128+m, k*128+p]
    owt = np.ascontiguousarray(
        o_w.reshape(16, 128, 16, 128).transpose(0, 3, 2, 1).reshape(16, 128, HID)
    ).astype(ml_dtypes.bfloat16)

    kvb = (kv_b_w * kv_norm_w[None, :]).reshape(H, 2, D, RANK)

    in_maps = []
    for c in range(NCORES):
        qwT = tf32_round(np.ascontiguousarray(
            (q_w[c * HPC * D:(c + 1) * HPC * D] * scale).T))           # [HID, 256]
        # kvbT rows order per core: k0,v0,k1,v1 each 128 wide
        blk = kvb[c * HPC:(c + 1) * HPC]                               # [2,2,128,RANK]
        kvbT = tf32_round(np.ascontiguousarray(
            blk.reshape(HPC * 2 * D, RANK).T))                         # [RANK, 512]
        hsmy = tf32_round(np.ascontiguousarray(
            hsT[:, c * TPC:(c + 1) * TPC]))
        in_maps.append({
            "hsT": hsT, "hsmy": hsmy, "qwT": qwT, "kvaT": kvaT,
            "kvbT": kvbT, "owt": owt, "cos": cosT, "sinsh": sinsh,
            "ident": ident, "ones": ones,
        })
    return in_maps


def assemble_output(results):
    out = np.empty((B, S, HID), dtype=np.float32)
    for c in range(NCORES):
        r = results[c]["outT"]                 # [HID, 2*SPC]
        out[0, c * SPC:(c + 1) * SPC, :] = r[:, 0:SPC].T
        out[1, c * SPC:(c + 1) * SPC, :] = r[:, SPC:2 * SPC].T
    return out


def kernel(hidden_states, q_w, kv_a_w, kv_b_w, o_w, kv_norm_w):
    from concourse import bass_utils

    if "nc" not in _CACHE:
        _CACHE["nc"] = build_nc()
    nc = _CACHE["nc"]
    in_maps = build_in_maps(hidden_states, q_w, kv_a_w, kv_b_w, o_w, kv_norm_w)
    res = bass_utils.run_bass_kernel_spmd(
        nc, in_maps, core_ids=list(range(NCORES)), trace=False)
    return assemble_output(res.results)

